# revision 24
# baseline (speedup 1.0000x reference)
"""Conv2d 3x3 (stride 1, pad 1) + bias on Trainium2, data-parallel over batch.

Full problem: x [32,128,56,56] f32, filters [256,128,3,3], biases [256]
-> out [32,256,56,56].  8 NeuronCores, 4 images per core.

Per-core kernel: conv as 9 shifted matmuls accumulated in PSUM.
  - contraction K = C_in = 128 (partition dim, exact fit)
  - stationary  = filter tap slice [128 cin, 128 cout]  (C_out=256 -> 2 halves)
  - moving      = padded input rows [128 cin, 8 rows x 56 cols = 448]
  - fp16 operands (values are ~N(0,1) -- no range risk): 1 cycle/row on
    the PE, Fast Weight Load (4x) hides the per-tap weight reload, and
    input DMA bytes halve.  fp32 PSUM accumulation keeps the error at
    ~2.7e-4 (measured vs the fp32 reference; bf16 would be 2.2e-3).
x is loaded in row-chunks (with 2-row halo overlap) so the first matmuls
start after ~0.5 MB of DMA instead of the full 6.9 MB.  Input DMAs ride the
SP HWDGE queue; the bias-add runs on ACT which then triggers output DMAs on
its own HWDGE queue (same-engine FIFO ordering, separate HW queues).
Host-side prep is layout only: zero-pad x to 58x58, transpose filters to
[cin, tap, cout], fold biases to [128, 2].
"""

import numpy as np

import concourse.bass as bass
import concourse.mybir as mybir
import concourse.tile as tile
from concourse import bacc
from concourse.bass_utils import run_bass_kernel_spmd

NCORES = 8
B, CIN, H, W = 32, 128, 56, 56
COUT, F = 256, 3
BLOC = B // NCORES  # 4 images per core
HP, WP = H + 2, W + 2  # 58x58 padded
RG = 8  # output rows per matmul group
NGRP = H // RG  # 7 row groups
NMOV = RG * W  # 448 moving elements per matmul

# x row-chunks (padded-row ranges, inclusive start / exclusive end); chunk c
# covers the halo rows for the groups listed in CHUNK_GROUPS[c].
CHUNKS = [(0, 18), (16, 34), (32, 50), (48, 58)]
CHUNK_OF_GROUP = {0: 0, 1: 0, 2: 1, 3: 1, 4: 2, 5: 2, 6: 3}

F32 = mybir.dt.float32
F16 = mybir.dt.float16

_CACHE = {}


def _build_nc():
    nc = bacc.Bacc("TRN2", target_bir_lowering=False, debug=False,
                   num_devices=NCORES)
    xp_d = nc.dram_tensor("xp", [BLOC, CIN, HP, WP], F16,
                          kind="ExternalInput").ap()
    wt_d = nc.dram_tensor("wt", [CIN, F * F * COUT], F16,
                          kind="ExternalInput").ap()
    bias_d = nc.dram_tensor("bias", [128, 2], F32, kind="ExternalInput").ap()
    out_d = nc.dram_tensor("out", [BLOC, COUT, H, W], F32,
                           kind="ExternalOutput").ap()

    with tile.TileContext(nc) as tc:
        with (
            tc.tile_pool(name="weights", bufs=1) as wpool,
            tc.tile_pool(name="xin", bufs=1) as xpool,
            tc.tile_pool(name="outs", bufs=4) as opool,
            tc.tile_pool(name="psum", bufs=8, space="PSUM") as ppool,
        ):
            # PE warm-up: the HAM clock gate keeps the PE at 1.2 GHz until
            # it has seen ~3.4us of sustained activity.  Burn that window on
            # dummy matmuls over a zeroed tile while the input DMAs stream,
            # so every real matmul runs at 2.4 GHz.
            warm = wpool.tile([CIN, NMOV], F16, name="warm")
            nc.gpsimd.memset(warm[:], 0.0)
            wps = ppool.tile([128, NMOV], F32, name="wps", tag="ps")
            for _ in range(16):
                nc.tensor.matmul(wps[:], warm[:, :128], warm[:],
                                 start=True, stop=True)

            # First x chunk of batch 0 goes first so compute starts ASAP.
            xtiles = {}

            def load_chunk(b, c):
                r0, r1 = CHUNKS[c]
                xt = xpool.tile([CIN, (r1 - r0) * WP], F16,
                                name=f"x{b}c{c}")
                nc.sync.dma_start(
                    xt[:], xp_d[b, :, r0:r1, :].rearrange("c h w -> c (h w)"))
                xtiles[(b, c)] = xt

            # Weight taps stream one 64 KB DMA each so the first matmul only
            # gates on tap 0 + the first x chunk, not the whole 0.6 MB.
            wt_sb = wpool.tile([CIN, F * F * COUT], F16, name="wt_sb")
            nc.sync.dma_start(wt_sb[:, 0:COUT], wt_d[:, 0:COUT])
            load_chunk(0, 0)
            for t in range(1, F * F):
                nc.sync.dma_start(wt_sb[:, t * COUT:(t + 1) * COUT],
                                  wt_d[:, t * COUT:(t + 1) * COUT])
            bias_sb = wpool.tile([128, 2], F32, name="bias_sb")
            nc.sync.dma_start(bias_sb[:], bias_d[:])
            load_chunk(0, 1)
            for b in range(BLOC):
                for c in range(len(CHUNKS)):
                    if (b, c) not in ((0, 0), (0, 1)):
                        load_chunk(b, c)

            # Groups are processed in pairs so each output DMA moves 16 rows
            # (459 KB, 3584 B/partition chunks) instead of 8 — larger chunks
            # drain the HBM write queues faster.  DMAs alternate 2:1 between
            # the ACT and SP HWDGE queues (SP also carries the input loads).
            GPAIRS = [(0, 1), (2, 3), (4, 5), (6,)]
            ndma = 0
            for b in range(BLOC):
                for pair in GPAIRS:
                    for half in range(2):
                        prows = len(pair) * RG
                        ot = opool.tile([128, prows * W], F32, name="ot")
                        for gi, g in enumerate(pair):
                            c = CHUNK_OF_GROUP[g]
                            r0 = CHUNKS[c][0]
                            nrows = CHUNKS[c][1] - r0
                            xv = xtiles[(b, c)][:].rearrange(
                                "c (h w) -> c h w", h=nrows)
                            ps = ppool.tile([128, NMOV], F32, name="ps")
                            for t in range(F * F):
                                dy, dx = divmod(t, F)
                                lr = g * RG + dy - r0
                                rhs = xv[:, lr: lr + RG, dx: dx + W]
                                lhsT = wt_sb[:, t * COUT + half * 128:
                                             t * COUT + half * 128 + 128]
                                nc.tensor.matmul(
                                    ps[:], lhsT, rhs,
                                    start=(t == 0), stop=(t == F * F - 1))
                            nc.scalar.add(
                                ot[:, gi * NMOV:(gi + 1) * NMOV], ps[:],
                                bias_sb[:, half: half + 1])
                        dst = out_d[b, half * 128: half * 128 + 128,
                                    pair[0] * RG: pair[0] * RG + prows, :]
                        eng = nc.sync if (ndma % 3 == 2) else nc.scalar
                        ndma += 1
                        eng.dma_start(
                            dst.rearrange("o h w -> o (h w)"), ot[:])
    # Bacc passes: split multi-waits into event-semaphore chains (HW allows
    # at most one sync wait per instruction), move matmul waits to ldweights.
    nc.compile()
    return nc


def _get_nc():
    if "nc" not in _CACHE:
        _CACHE["nc"] = _build_nc()
    return _CACHE["nc"]


def _prep(x, filters, biases):
    xp = np.zeros((B, CIN, HP, WP), np.float16)
    xp[:, :, 1:1 + H, 1:1 + W] = x.astype(np.float16)
    wt = np.ascontiguousarray(
        filters.transpose(1, 2, 3, 0)).reshape(CIN, F * F * COUT)
    wt = wt.astype(np.float16)
    bias2 = np.ascontiguousarray(biases.reshape(2, 128).T)
    return xp, wt, bias2


def kernel(x, filters, biases):
    x = np.ascontiguousarray(x, dtype=np.float32)
    filters = np.ascontiguousarray(filters, dtype=np.float32)
    biases = np.ascontiguousarray(biases, dtype=np.float32)

    xp, wt, bias2 = _prep(x, filters, biases)
    nc = _get_nc()
    in_maps = [
        {"xp": xp[c * BLOC: (c + 1) * BLOC], "wt": wt, "bias": bias2}
        for c in range(NCORES)
    ]
    res = run_bass_kernel_spmd(nc, in_maps, list(range(NCORES)))
    out = np.concatenate([res.results[c]["out"] for c in range(NCORES)],
                         axis=0)
    return out
